# revision 5
# baseline (speedup 1.0000x reference)
"""Trainium2 Bass kernel for nn_DiffFDN: H(e^jw) = C (D(w) - A Gamma)^-1 B
plus h = irfft(sum_ch H) / max|.|, distributed over 8 NeuronCores.

Self-contained: host-side prep (16x16 expm, constant DFT tables), an SPMD
Bass/Tile kernel (Neumann-iteration batched solve + matmul FFT), and
gather/unshard logic.

Per core: 24064 frequencies, partition layout p = 16*g + ch (8 groups x 16
channels), free dim j in [0,3008), f_local = 3008*g + j.  The 16x16 complex
systems (D - AG) z = B are solved by the fixed-point iteration
    z <- Dinv * (B + AG z)      (AG real, |rho| ~ 0.774)
realized as one block-diagonal 128x128 TensorE matmul + 6 VectorE ops per
512-chunk per iteration.  Hs = sum_ch C*z via matmul, AllGather, then every
core redundantly synthesizes h = irfft(Hs) via a half-size complex iFFT
(192000 = 384*500) done as PE matmuls, and normalizes by max|h|.
"""
import os
import numpy as np
from contextlib import ExitStack

import concourse.bass as bass
import concourse.bacc as bacc
import concourse.mybir as mybir
import concourse.tile as tile
from concourse.bass_utils import run_bass_kernel_spmd

dt = mybir.dt
AF = mybir.ActivationFunctionType
ALU = mybir.AluOpType
AX = mybir.AxisListType

# ---------------- problem constants ----------------
N = 16
F = 192001
NFFT = 384000
NH = 192000          # NFFT // 2
N1, N2 = 384, 500    # NH = N1*N2
NCORES = 8
FSTEP = 24000        # valid frequencies per core (last core: 24001)
FC = 24064           # computed frequencies per core = 8*3008
FG = 3008            # frequencies per partition-group
NGRP = 8
KITER = int(os.environ.get("DFDN_K", "40"))
CHUNK = 512
CHUNKS = [(i * CHUNK, min(CHUNK, FG - i * CHUNK)) for i in range((FG + CHUNK - 1) // CHUNK)]

M_DELAYS = np.array([809., 877., 937., 1049., 1151., 1249., 1373., 1499.,
                     1617., 1753., 1879., 2003., 2131., 2269., 2393., 2521.],
                    np.float32)
# exact fp32 gamma values as produced by the reference (jax fp32 0.9998**m)
GAMMA_F32 = np.array([0.85061574, 0.83912605, 0.829117, 0.81075186, 0.7943806,
                      0.7789628, 0.75988275, 0.74097353, 0.7236918, 0.70427334,
                      0.68674797, 0.6699266, 0.6529947, 0.63521904, 0.6196598,
                      0.60399836], np.float32)


def _split3(x64):
    """split x into 3 fp32 parts, hi/mid with 12 zeroed mantissa tail bits"""
    def trunc(v):
        return np.frombuffer((np.frombuffer(np.float32(v).tobytes(), np.uint32)
                              & np.uint32(0xFFFFF000)).tobytes(), np.float32)[0]
    hi = trunc(np.float32(x64))
    mid = trunc(np.float32(x64 - np.float64(hi)))
    lo = np.float32(x64 - np.float64(hi) - np.float64(mid))
    return float(hi), float(mid), float(lo)


CW0, CW1, CW2 = _split3(2 * np.pi)
MAGIC = float(np.float32(1.5 * 2 ** 23))
INV2PI = float(np.float32(1.0 / (2 * np.pi)))


def _expm_skew(X64):
    """expm(triu(X,1) - triu(X,1)^T) via Pade-13 scaling & squaring, float64"""
    Au = np.triu(X64, 1)
    A = Au - Au.T
    nrm = np.linalg.norm(A, 1)
    j = max(0, int(np.ceil(np.log2(max(nrm / 5.4, 2.0 ** -60)))))
    A = A / (2.0 ** j)
    b = [64764752532480000., 32382376266240000., 7771770303897600.,
         1187353796428800., 129060195264000., 10559470521600., 670442572800.,
         33522128640., 1323241920., 40840800., 960960., 16380., 182., 1.]
    I = np.eye(A.shape[0])
    A2 = A @ A; A4 = A2 @ A2; A6 = A2 @ A4
    U = A @ (A6 @ (b[13] * A6 + b[11] * A4 + b[9] * A2)
             + b[7] * A6 + b[5] * A4 + b[3] * A2 + b[1] * I)
    V = (A6 @ (b[12] * A6 + b[10] * A4 + b[8] * A2)
         + b[6] * A6 + b[4] * A4 + b[2] * A2 + b[0] * I)
    E = np.linalg.solve(V - U, V + U)
    for _ in range(j):
        E = E @ E
    return E


def _dap(t, offset, pairs):
    return bass.AP(tensor=t.tensor, offset=offset, ap=[list(p) for p in pairs])


# ---------------- device program ----------------
_NC_CACHE = {}
_LAST = {}


def _build_nc():
    if "nc" in _NC_CACHE:
        return _NC_CACHE["nc"]
    nc = bacc.Bacc("TRN2", target_bir_lowering=False, debug=False,
                   num_devices=NCORES)

    def din(name, shape):
        return nc.dram_tensor(name, shape, dt.float32, kind="ExternalInput").ap()

    def dout(name, shape):
        return nc.dram_tensor(name, shape, dt.float32, kind="ExternalOutput").ap()

    # per-core sharded inputs
    theta_in = din("theta_in", [FC])      # atan2(x_im, x_re) for this shard
    lh_in = din("lh_in", [FC])            # log|x| for this shard
    # shared constants
    m_col = din("m_col", [128, 1])
    negm_col = din("negm_col", [128, 1])
    b_col = din("b_col", [128, 1])
    c_col = din("c_col", [128, 1])
    w_ag = din("w_ag", [128, 128])        # blockdiag8(AG^T)
    w_sum = din("w_sum", [128, 8])        # channel-sum weights
    jrev = din("jrev", [128, 128])        # partition reversal permutation
    t2r_in = din("t2r", [512, N2])
    t2i_in = din("t2i", [512, N2])
    t2ni_in = din("t2ni", [512, N2])
    t1r_in = din("t1r", [N1, N1])
    t1i_in = din("t1i", [N1, N1])
    t1ni_in = din("t1ni", [N1, N1])
    twr_in = din("twr", [N1, N2])
    twi_in = din("twi", [N1, N2])
    tbr_in = din("tbr", [NH])
    tbi_in = din("tbi", [NH])
    # outputs
    out_hre = dout("out_hre", [128, FG])
    out_him = dout("out_him", [128, FG])
    out_h = dout("out_h", [NFFT])
    # internal DRAM
    z_re_d = nc.dram_tensor("z_re_d", [NH], dt.float32).ap()
    z_im_d = nc.dram_tensor("z_im_d", [NH], dt.float32).ap()
    hs_full_re = nc.dram_tensor("hs_full_re", [F], dt.float32).ap()
    hs_full_im = nc.dram_tensor("hs_full_im", [F], dt.float32).ap()
    scr = nc.dram_tensor("scr", [128], dt.float32).ap()

    with tile.TileContext(nc) as tc, ExitStack() as ctx:
        dpool = ctx.enter_context(tc.tile_pool(name="dram", bufs=1, space="DRAM"))
        hs_shard = dpool.tile([2, FC], dt.float32)
        hs_gath = dpool.tile([NCORES, 2, FC], dt.float32, addr_space="Shared")

        cpool = ctx.enter_context(tc.tile_pool(name="consts", bufs=1))
        mc = cpool.tile([128, 1], dt.float32, tag="mc")
        nmc = cpool.tile([128, 1], dt.float32, tag="nmc")
        bc = cpool.tile([128, 1], dt.float32, tag="bc")
        cc = cpool.tile([128, 1], dt.float32, tag="cc")
        hpi = cpool.tile([128, 1], dt.float32, tag="hpi")
        wag = cpool.tile([128, 128], dt.float32, tag="wag")
        wsum = cpool.tile([128, 8], dt.float32, tag="wsum")
        nc.sync.dma_start(mc[:], m_col[:])
        nc.sync.dma_start(nmc[:], negm_col[:])
        nc.sync.dma_start(bc[:], b_col[:])
        nc.sync.dma_start(cc[:], c_col[:])
        nc.vector.memset(hpi[:], float(np.pi / 2))
        nc.sync.dma_start(wag[:], w_ag[:])
        nc.sync.dma_start(wsum[:], w_sum[:])

        spool = ctx.enter_context(tc.tile_pool(name="solve", bufs=1))
        ar = spool.tile([128, FG], dt.float32, tag="ar")    # Dinv real
        bi = spool.tile([128, FG], dt.float32, tag="bi")    # Dinv imag
        z0r = spool.tile([128, FG], dt.float32, tag="z0r")
        z0i = spool.tile([128, FG], dt.float32, tag="z0i")
        z1r = spool.tile([128, FG], dt.float32, tag="z1r")
        z1i = spool.tile([128, FG], dt.float32, tag="z1i")

        # ---- phase precompute ----
        with tc.tile_pool(name="pre", bufs=1) as pp:
            th = pp.tile([128, FG], dt.float32, tag="th")
            lht = pp.tile([128, FG], dt.float32, tag="lht")
            for g in range(NGRP):
                nc.sync.dma_start(th[16 * g:16 * (g + 1), :],
                                  _dap(theta_in, g * FG, [[0, 16], [1, FG]]))
                nc.sync.dma_start(lht[16 * g:16 * (g + 1), :],
                                  _dap(lh_in, g * FG, [[0, 16], [1, FG]]))
            with tc.tile_pool(name="prechunk", bufs=3) as pc:
                # pass A: -sin(phi) -> z0r, cos(phi) -> z0i
                for c0, cl in CHUNKS:
                    sl = slice(c0, c0 + cl)
                    phi = pc.tile([128, CHUNK], dt.float32, tag="w0")
                    nc.vector.tensor_scalar(phi[:, :cl], th[:, sl], mc[:], None,
                                            op0=ALU.mult)
                    kk = pc.tile([128, CHUNK], dt.float32, tag="w1")
                    nc.vector.tensor_scalar(kk[:, :cl], phi[:, :cl], INV2PI,
                                            MAGIC, op0=ALU.mult, op1=ALU.add)
                    nc.vector.tensor_scalar_sub(kk[:, :cl], kk[:, :cl], MAGIC)
                    rr = pc.tile([128, CHUNK], dt.float32, tag="w2")
                    nc.vector.cody_waite_cascade(rr[:, :cl], phi[:, :cl],
                                                 kk[:, :cl], CW0, CW1, CW2)
                    nc.vector.add_range_wrap(rr[:, :cl], rr[:, :cl], 0.0,
                                             float(np.pi), float(2 * np.pi))
                    nc.scalar.activation(z0r[:, sl], rr[:, :cl], AF.Sin,
                                         scale=-1.0)          # -sin(phi)
                    aa = pc.tile([128, CHUNK], dt.float32, tag="w3")
                    nc.scalar.activation(aa[:, :cl], rr[:, :cl], AF.Abs)
                    nc.scalar.activation(z0i[:, sl], aa[:, :cl], AF.Sin,
                                         bias=hpi[:], scale=-1.0)  # cos(phi)
                # pass B: rhoinv = exp(-m*lh); ar = cos*rhoinv; bi = -sin*rhoinv
                for c0, cl in CHUNKS:
                    sl = slice(c0, c0 + cl)
                    rho = pc.tile([128, CHUNK], dt.float32, tag="w0")
                    nc.scalar.activation(rho[:, :cl], lht[:, sl], AF.Exp,
                                         scale=nmc[:])
                    nc.vector.tensor_mul(ar[:, sl], z0i[:, sl], rho[:, :cl])
                    nc.vector.tensor_mul(bi[:, sl], z0r[:, sl], rho[:, :cl])

        # ---- Neumann iteration: z_t = Dinv*(B + AG z_{t-1}), z_0 = Dinv*B ----
        nc.vector.tensor_scalar(z0r[:], ar[:], bc[:], None, op0=ALU.mult)
        nc.vector.tensor_scalar(z0i[:], bi[:], bc[:], None, op0=ALU.mult)
        zs = [(z0r, z0i), (z1r, z1i)]
        with tc.tile_pool(name="nps", bufs=2, space="PSUM") as nps, \
             tc.tile_pool(name="nwk", bufs=3) as nwk:
            for t in range(1, KITER):
                cur, nxt = zs[(t + 1) % 2], zs[t % 2]
                for c0, cl in CHUNKS:
                    sl = slice(c0, c0 + cl)
                    ur = nps.tile([128, CHUNK], dt.float32, tag="ur")
                    ui = nps.tile([128, CHUNK], dt.float32, tag="ui")
                    nc.tensor.matmul(ur[:, :cl], lhsT=wag[:], rhs=cur[0][:, sl],
                                     start=True, stop=True)
                    nc.tensor.matmul(ui[:, :cl], lhsT=wag[:], rhs=cur[1][:, sl],
                                     start=True, stop=True)
                    p = nwk.tile([128, CHUNK], dt.float32, tag="p")
                    q = nwk.tile([128, CHUNK], dt.float32, tag="q")
                    s = nwk.tile([128, CHUNK], dt.float32, tag="s")
                    t2 = nwk.tile([128, CHUNK], dt.float32, tag="t2")
                    # p=(ur+B)*ar ; t2=(ur+B)*bi ; q=bi*ui ; s=ar*ui
                    nc.vector.scalar_tensor_tensor(p[:, :cl], ur[:, :cl], bc[:],
                                                   ar[:, sl], op0=ALU.add,
                                                   op1=ALU.mult)
                    nc.vector.scalar_tensor_tensor(t2[:, :cl], ur[:, :cl], bc[:],
                                                   bi[:, sl], op0=ALU.add,
                                                   op1=ALU.mult)
                    nc.vector.tensor_mul(q[:, :cl], bi[:, sl], ui[:, :cl])
                    nc.vector.tensor_mul(s[:, :cl], ar[:, sl], ui[:, :cl])
                    nc.vector.tensor_sub(nxt[0][:, sl], p[:, :cl], q[:, :cl])
                    nc.vector.tensor_add(nxt[1][:, sl], s[:, :cl], t2[:, :cl])

        zfr, zfi = zs[(KITER - 1) % 2]

        # ---- H = C*z out; Hs = sum_ch H via matmul ----
        hs_re = spool.tile([8, FG], dt.float32, tag="hsr")
        hs_im = spool.tile([8, FG], dt.float32, tag="hsi")
        with tc.tile_pool(name="hps", bufs=2, space="PSUM") as hps, \
             tc.tile_pool(name="hwk", bufs=3) as hwk:
            for c0, cl in CHUNKS:
                sl = slice(c0, c0 + cl)
                hr = hwk.tile([128, CHUNK], dt.float32, tag="hr")
                hi = hwk.tile([128, CHUNK], dt.float32, tag="hi")
                nc.vector.tensor_scalar(hr[:, :cl], zfr[:, sl], cc[:], None,
                                        op0=ALU.mult)
                nc.vector.tensor_scalar(hi[:, :cl], zfi[:, sl], cc[:], None,
                                        op0=ALU.mult)
                nc.sync.dma_start(out_hre[:, sl], hr[:, :cl])
                nc.sync.dma_start(out_him[:, sl], hi[:, :cl])
                pr = hps.tile([8, CHUNK], dt.float32, tag="pr")
                pi = hps.tile([8, CHUNK], dt.float32, tag="pi")
                nc.tensor.matmul(pr[:, :cl], lhsT=wsum[:], rhs=hr[:, :cl],
                                 start=True, stop=True)
                nc.tensor.matmul(pi[:, :cl], lhsT=wsum[:], rhs=hi[:, :cl],
                                 start=True, stop=True)
                nc.vector.tensor_copy(hs_re[:, sl], pr[:, :cl])
                nc.vector.tensor_copy(hs_im[:, sl], pi[:, :cl])
        # store shard [2, FC]: row-major (g, j)
        nc.sync.dma_start(hs_shard[0].rearrange("(g j) -> g j", g=8), hs_re[:, :])
        nc.sync.dma_start(hs_shard[1].rearrange("(g j) -> g j", g=8), hs_im[:, :])

        # ---- AllGather + compaction ----
        nc.gpsimd.collective_compute(
            "AllGather", ALU.bypass, replica_groups=[list(range(NCORES))],
            ins=[hs_shard[:]], outs=[hs_gath[:]],
        )
        for c in range(NCORES):
            ln = FSTEP if c < NCORES - 1 else FSTEP + 1
            nc.sync.dma_start(hs_full_re[FSTEP * c:FSTEP * c + ln],
                              hs_gath[c, 0, :ln])
            nc.sync.dma_start(hs_full_im[FSTEP * c:FSTEP * c + ln],
                              hs_gath[c, 1, :ln])

        # ---- G build: Z[k] = E[k] + i O[k] (scaled by 2; scale cancels) ----
        with tc.tile_pool(name="gb", bufs=1) as gb, \
             tc.tile_pool(name="gps", bufs=1, space="PSUM") as gps:
            jr = gb.tile([128, 128], dt.float32, tag="jr")
            nc.sync.dma_start(jr[:], jrev[:])
            xr = gb.tile([128, 1500], dt.float32, tag="xr")
            xi = gb.tile([128, 1500], dt.float32, tag="xi")
            tbrt = gb.tile([128, 1500], dt.float32, tag="tbrt")
            tbit = gb.tile([128, 1500], dt.float32, tag="tbit")
            nc.sync.dma_start(xr[:], _dap(hs_full_re, 0, [[1500, 128], [1, 1500]]))
            nc.sync.dma_start(xi[:], _dap(hs_full_im, 0, [[1500, 128], [1, 1500]]))
            nc.sync.dma_start(tbrt[:], _dap(tbr_in, 0, [[1500, 128], [1, 1500]]))
            nc.sync.dma_start(tbit[:], _dap(tbi_in, 0, [[1500, 128], [1, 1500]]))
            # reversed reads: tmp[p,j] = X[1500p + 1500 - j]; rev = J @ tmp
            tmpr = gb.tile([128, 1500], dt.float32, tag="tmpr")
            tmpi = gb.tile([128, 1500], dt.float32, tag="tmpi")
            nc.sync.dma_start(tmpr[:], _dap(hs_full_re, 1500,
                                            [[1500, 128], [-1, 1500]]))
            nc.sync.dma_start(tmpi[:], _dap(hs_full_im, 1500,
                                            [[1500, 128], [-1, 1500]]))
            rvr = gps.tile([128, 1500], dt.float32, tag="rvr")
            rvi = gps.tile([128, 1500], dt.float32, tag="rvi")
            for c0 in range(0, 1500, 512):
                cl = min(512, 1500 - c0)
                nc.tensor.matmul(rvr[:, c0:c0 + cl], lhsT=jr[:],
                                 rhs=tmpr[:, c0:c0 + cl], start=True, stop=True)
                nc.tensor.matmul(rvi[:, c0:c0 + cl], lhsT=jr[:],
                                 rhs=tmpi[:, c0:c0 + cl], start=True, stop=True)
            er = gb.tile([128, 1500], dt.float32, tag="er")
            ei = gb.tile([128, 1500], dt.float32, tag="ei")
            opr = gb.tile([128, 1500], dt.float32, tag="opr")
            opi = gb.tile([128, 1500], dt.float32, tag="opi")
            nc.vector.tensor_add(er[:], xr[:], rvr[:])
            nc.vector.tensor_sub(ei[:], xi[:], rvi[:])
            nc.vector.tensor_sub(opr[:], xr[:], rvr[:])
            nc.vector.tensor_add(opi[:], xi[:], rvi[:])
            our = gb.tile([128, 1500], dt.float32, tag="our")
            oui = gb.tile([128, 1500], dt.float32, tag="oui")
            tq = gb.tile([128, 1500], dt.float32, tag="tq")
            nc.vector.tensor_mul(our[:], tbrt[:], opr[:])
            nc.vector.tensor_mul(tq[:], tbit[:], opi[:])
            nc.vector.tensor_sub(our[:], our[:], tq[:])
            nc.vector.tensor_mul(oui[:], tbrt[:], opi[:])
            nc.vector.tensor_mul(tq[:], tbit[:], opr[:])
            nc.vector.tensor_add(oui[:], oui[:], tq[:])
            nc.vector.tensor_sub(er[:], er[:], oui[:])   # Z_re = E_re - O_im
            nc.vector.tensor_add(ei[:], ei[:], our[:])   # Z_im = E_im + O_re
            nc.sync.dma_start(_dap(z_re_d, 0, [[1500, 128], [1, 1500]]), er[:])
            nc.sync.dma_start(_dap(z_im_d, 0, [[1500, 128], [1, 1500]]), ei[:])

        # ---- FFT stage 2 + twiddle;  Y1[a,d] = sum_b Zmat[a,b] T2[b,d] ----
        BCH = [(0, 128), (128, 128), (256, 128), (384, 116)]
        with tc.tile_pool(name="y2p", bufs=1) as y2p:
            y2 = [(y2p.tile([128, N2], dt.float32, tag=f"y2r{i}", name=f"y2r{i}"),
                   y2p.tile([128, N2], dt.float32, tag=f"y2i{i}", name=f"y2i{i}"))
                  for i in range(3)]
            with tc.tile_pool(name="f2c", bufs=1) as f2c, \
                 tc.tile_pool(name="f2w", bufs=2) as f2w, \
                 tc.tile_pool(name="f2ps", bufs=2, space="PSUM") as f2ps:
                t2t = {}
                for ib, (b0, bn) in enumerate(BCH):
                    for nm, src in (("r", t2r_in), ("i", t2i_in), ("ni", t2ni_in)):
                        tt_ = f2c.tile([128, N2], dt.float32, tag=f"t2{nm}{ib}",
                                       name=f"t2{nm}{ib}")
                        nc.sync.dma_start(tt_[:bn, :], src[b0:b0 + bn, :])
                        t2t[nm, ib] = tt_
                for ac in range(3):
                    y1r = f2ps.tile([128, N2], dt.float32, tag="y1r")
                    y1i = f2ps.tile([128, N2], dt.float32, tag="y1i")
                    for ib, (b0, bn) in enumerate(BCH):
                        zw_r = f2w.tile([128, 128], dt.float32, tag="zwr")
                        zw_i = f2w.tile([128, 128], dt.float32, tag="zwi")
                        nc.sync.dma_start(zw_r[:bn, :],
                                          _dap(z_re_d, 128 * ac + N1 * b0,
                                               [[N1, bn], [1, 128]]))
                        nc.sync.dma_start(zw_i[:bn, :],
                                          _dap(z_im_d, 128 * ac + N1 * b0,
                                               [[N1, bn], [1, 128]]))
                        st = (ib == 0)
                        sp = (ib == len(BCH) - 1)
                        nc.tensor.matmul(y1r[:], lhsT=zw_r[:bn, :],
                                         rhs=t2t["r", ib][:bn, :],
                                         start=st, stop=False)
                        nc.tensor.matmul(y1r[:], lhsT=zw_i[:bn, :],
                                         rhs=t2t["ni", ib][:bn, :],
                                         start=False, stop=sp)
                        nc.tensor.matmul(y1i[:], lhsT=zw_r[:bn, :],
                                         rhs=t2t["i", ib][:bn, :],
                                         start=st, stop=False)
                        nc.tensor.matmul(y1i[:], lhsT=zw_i[:bn, :],
                                         rhs=t2t["r", ib][:bn, :],
                                         start=False, stop=sp)
                    twr_t = f2w.tile([128, N2], dt.float32, tag="twr")
                    twi_t = f2w.tile([128, N2], dt.float32, tag="twi")
                    nc.sync.dma_start(twr_t[:], twr_in[128 * ac:128 * (ac + 1), :])
                    nc.sync.dma_start(twi_t[:], twi_in[128 * ac:128 * (ac + 1), :])
                    y2r, y2i = y2[ac]
                    tq1 = f2w.tile([128, N2], dt.float32, tag="tq1")
                    tq2 = f2w.tile([128, N2], dt.float32, tag="tq2")
                    nc.vector.tensor_mul(tq1[:], twr_t[:], y1r[:])
                    nc.vector.tensor_mul(tq2[:], twi_t[:], y1i[:])
                    nc.vector.tensor_sub(y2r[:], tq1[:], tq2[:])
                    nc.vector.tensor_mul(tq1[:], twr_t[:], y1i[:])
                    nc.vector.tensor_mul(tq2[:], twi_t[:], y1r[:])
                    nc.vector.tensor_add(y2i[:], tq1[:], tq2[:])

            # ---- FFT stage 4: z[c,d] = sum_a T1[a,c] Y2[a,d]; max; output ----
            with tc.tile_pool(name="f4c", bufs=2) as f4c, \
                 tc.tile_pool(name="f4ps", bufs=2, space="PSUM") as f4ps, \
                 tc.tile_pool(name="zzp", bufs=1) as zzp:
                zz = [(zzp.tile([128, N2], dt.float32, tag=f"zzr{i}", name=f"zzr{i}"),
                       zzp.tile([128, N2], dt.float32, tag=f"zzi{i}", name=f"zzi{i}"))
                      for i in range(3)]
                mxt = zzp.tile([128, 1], dt.float32, tag="mxt")
                mxc = zzp.tile([128, 1], dt.float32, tag="mxc")
                for cc_ in range(3):
                    zr_ps = f4ps.tile([128, N2], dt.float32, tag="zr")
                    zi_ps = f4ps.tile([128, N2], dt.float32, tag="zi")
                    for ac in range(3):
                        t1r_b = f4c.tile([128, 128], dt.float32, tag="t1r")
                        t1i_b = f4c.tile([128, 128], dt.float32, tag="t1i")
                        t1ni_b = f4c.tile([128, 128], dt.float32, tag="t1ni")
                        rsl = slice(128 * ac, 128 * (ac + 1))
                        csl = slice(128 * cc_, 128 * (cc_ + 1))
                        nc.sync.dma_start(t1r_b[:], t1r_in[rsl, csl])
                        nc.sync.dma_start(t1i_b[:], t1i_in[rsl, csl])
                        nc.sync.dma_start(t1ni_b[:], t1ni_in[rsl, csl])
                        st = (ac == 0)
                        sp = (ac == 2)
                        y2r, y2i = y2[ac]
                        nc.tensor.matmul(zr_ps[:], lhsT=t1r_b[:], rhs=y2r[:],
                                         start=st, stop=False)
                        nc.tensor.matmul(zr_ps[:], lhsT=t1ni_b[:], rhs=y2i[:],
                                         start=False, stop=sp)
                        nc.tensor.matmul(zi_ps[:], lhsT=t1i_b[:], rhs=y2r[:],
                                         start=st, stop=False)
                        nc.tensor.matmul(zi_ps[:], lhsT=t1r_b[:], rhs=y2i[:],
                                         start=False, stop=sp)
                    zzr, zzi = zz[cc_]
                    nc.vector.tensor_copy(zzr[:], zr_ps[:])
                    nc.vector.tensor_copy(zzi[:], zi_ps[:])
                    for ip, pl in enumerate((zzr, zzi)):
                        red = f4c.tile([128, 1], dt.float32, tag="red")
                        nc.vector.tensor_reduce(red[:], pl[:], axis=AX.X,
                                                op=ALU.max,
                                                apply_absolute_value=True)
                        if cc_ == 0 and ip == 0:
                            nc.vector.tensor_copy(mxt[:], red[:])
                        else:
                            nc.vector.tensor_max(mxt[:], mxt[:], red[:])
                # partition reduce via DRAM roundtrip, then broadcast 1/max
                nc.sync.dma_start(scr[:], mxt[:, 0])
                mrow = f4c.tile([1, 128], dt.float32, tag="mrow")
                nc.sync.dma_start(mrow[:], scr[None, :])
                m1 = f4c.tile([1, 1], dt.float32, tag="m1")
                nc.vector.tensor_reduce(m1[:], mrow[:], axis=AX.X, op=ALU.max)
                rc = f4c.tile([1, 1], dt.float32, tag="rc")
                nc.vector.reciprocal(rc[:], m1[:])
                nc.sync.dma_start(scr[:1], rc[:, 0])
                nc.sync.dma_start(mxc[:], _dap(scr, 0, [[0, 128], [1, 1]]))
                for cc_ in range(3):
                    zzr, zzi = zz[cc_]
                    nc.vector.tensor_scalar(zzr[:], zzr[:], mxc[:], None,
                                            op0=ALU.mult)
                    nc.vector.tensor_scalar(zzi[:], zzi[:], mxc[:], None,
                                            op0=ALU.mult)
                    nc.sync.dma_start(_dap(out_h, 2 * N2 * 128 * cc_,
                                           [[2 * N2, 128], [2, N2]]), zzr[:])
                    nc.sync.dma_start(_dap(out_h, 2 * N2 * 128 * cc_ + 1,
                                           [[2 * N2, 128], [2, N2]]), zzi[:])

    nc.compile()
    _NC_CACHE["nc"] = nc
    return nc


# ---------------- host side ----------------
def _host_constants(AG):
    f32 = np.float32
    c = {}
    ch = np.arange(128) % 16
    c["m_col"] = M_DELAYS[ch].reshape(128, 1).copy()
    c["negm_col"] = (-M_DELAYS[ch]).reshape(128, 1).copy()
    wag = np.zeros((128, 128), f32)
    for g in range(8):
        wag[16 * g:16 * (g + 1), 16 * g:16 * (g + 1)] = AG.T
    c["w_ag"] = wag
    wsum = np.zeros((128, 8), f32)
    for g in range(8):
        wsum[16 * g:16 * (g + 1), g] = 1.0
    c["w_sum"] = wsum
    c["jrev"] = np.eye(128, dtype=f32)[:, ::-1].copy()
    b, d = np.meshgrid(np.arange(512), np.arange(N2), indexing="ij")
    ang = 2 * np.pi * ((b * d) % N2) / N2
    mask = (b < N2)
    c["t2r"] = (np.cos(ang) * mask).astype(f32)
    c["t2i"] = (np.sin(ang) * mask).astype(f32)
    c["t2ni"] = (-np.sin(ang) * mask).astype(f32)
    a, cg = np.meshgrid(np.arange(N1), np.arange(N1), indexing="ij")
    ang = 2 * np.pi * ((a * cg) % N1) / N1
    c["t1r"] = (np.cos(ang) / NH).astype(f32)
    c["t1i"] = (np.sin(ang) / NH).astype(f32)
    c["t1ni"] = (-np.sin(ang) / NH).astype(f32)
    a, d = np.meshgrid(np.arange(N1), np.arange(N2), indexing="ij")
    ang = 2 * np.pi * (a.astype(np.float64) * d) / NH
    c["twr"] = np.cos(ang).astype(f32)
    c["twi"] = np.sin(ang).astype(f32)
    k = np.arange(NH)
    ang = 2 * np.pi * k / NFFT
    c["tbr"] = np.cos(ang).astype(f32)
    c["tbi"] = np.sin(ang).astype(f32)
    return c


def kernel(x_real, x_imag, B, C, X):
    f32 = np.float32
    x_real = np.asarray(x_real, f32)
    x_imag = np.asarray(x_imag, f32)
    A = _expm_skew(np.asarray(X, np.float64))
    AG = (A * GAMMA_F32.astype(np.float64)[None, :]).astype(f32)
    theta = np.arctan2(x_imag, x_real).astype(f32)
    lh = np.log(np.hypot(x_real, x_imag)).astype(f32)
    Bv = np.asarray(B, f32).reshape(N)
    Cv = np.asarray(C, f32).reshape(N)

    consts = _host_constants(AG)
    ch = np.arange(128) % 16
    consts["b_col"] = Bv[ch].reshape(128, 1).copy()
    consts["c_col"] = Cv[ch].reshape(128, 1).copy()

    idx = np.arange(FC)
    in_maps = []
    for c in range(NCORES):
        fidx = np.clip(FSTEP * c + idx, 0, F - 1)
        m = dict(consts)
        m["theta_in"] = theta[fidx].copy()
        m["lh_in"] = lh[fidx].copy()
        in_maps.append(m)

    nc = _build_nc()
    if os.environ.get("DFDN_SIM") == "1":
        from concourse.bass_interp import MultiCoreSim
        sim = MultiCoreSim(nc, num_cores=NCORES)
        for i in range(NCORES):
            for k, v in in_maps[i].items():
                sim.cores[i].tensor(k)[:] = v
        sim.simulate(check_with_hw=False)
        outs = ["out_hre", "out_him", "out_h"]
        results = [{nm: np.array(sim.cores[i].tensor(nm)) for nm in outs}
                   for i in range(NCORES)]

        class _R:
            pass
        br = _R()
        br.results = results
        br.exec_time_ns = None
    else:
        br = run_bass_kernel_spmd(nc, in_maps, core_ids=list(range(NCORES)),
                                  trace=os.environ.get("DFDN_TRACE") == "1")
    _LAST["br"] = br

    H = np.empty((F, N), np.complex64)
    for c in range(NCORES):
        ln = FSTEP if c < NCORES - 1 else FSTEP + 1
        hr = br.results[c]["out_hre"].reshape(8, 16, FG).transpose(0, 2, 1).reshape(FC, 16)
        hi = br.results[c]["out_him"].reshape(8, 16, FG).transpose(0, 2, 1).reshape(FC, 16)
        H[FSTEP * c:FSTEP * c + ln] = (hr[:ln] + 1j * hi[:ln]).astype(np.complex64)
    h = br.results[0]["out_h"].astype(f32)
    return H, h


# revision 6
# speedup vs baseline: 3.8112x; 3.8112x over previous
"""Trainium2 Bass kernel for nn_DiffFDN: H(e^jw) = C (D(w) - A Gamma)^-1 B
plus h = irfft(sum_ch H) / max|.|, distributed over 8 NeuronCores.

Self-contained: host-side prep (16x16 expm, constant DFT tables), an SPMD
Bass/Tile kernel (Neumann-iteration batched solve + matmul FFT), and
gather/unshard logic.

Per core: 24064 frequencies, partition layout p = 16*g + ch (8 groups x 16
channels), free dim j in [0,3008), f_local = 3008*g + j.  The 16x16 complex
systems (D - AG) z = B are solved by the fixed-point iteration
    z <- Dinv * (B + AG z)      (AG real, |rho| ~ 0.774)
realized as one block-diagonal 128x128 TensorE matmul + 6 VectorE ops per
512-chunk per iteration.  Hs = sum_ch C*z via matmul, AllGather, then every
core redundantly synthesizes h = irfft(Hs) via a half-size complex iFFT
(192000 = 384*500) done as PE matmuls, and normalizes by max|h|.
"""
import os
import numpy as np
from contextlib import ExitStack

import concourse.bass as bass
import concourse.bacc as bacc
import concourse.mybir as mybir
import concourse.tile as tile
from concourse.bass_utils import run_bass_kernel_spmd

dt = mybir.dt
AF = mybir.ActivationFunctionType
ALU = mybir.AluOpType
AX = mybir.AxisListType

# ---------------- problem constants ----------------
N = 16
F = 192001
NFFT = 384000
NH = 192000          # NFFT // 2
N1, N2 = 384, 500    # NH = N1*N2
NCORES = 8
FSTEP = 24000        # valid frequencies per core (last core: 24001)
FC = 24064           # computed frequencies per core = 8*3008
FG = 3008            # frequencies per partition-group
NGRP = 8
KITER = int(os.environ.get("DFDN_K", "40"))
CHUNK = 512
CHUNKS = [(i * CHUNK, min(CHUNK, FG - i * CHUNK)) for i in range((FG + CHUNK - 1) // CHUNK)]

M_DELAYS = np.array([809., 877., 937., 1049., 1151., 1249., 1373., 1499.,
                     1617., 1753., 1879., 2003., 2131., 2269., 2393., 2521.],
                    np.float32)
# exact fp32 gamma values as produced by the reference (jax fp32 0.9998**m)
GAMMA_F32 = np.array([0.85061574, 0.83912605, 0.829117, 0.81075186, 0.7943806,
                      0.7789628, 0.75988275, 0.74097353, 0.7236918, 0.70427334,
                      0.68674797, 0.6699266, 0.6529947, 0.63521904, 0.6196598,
                      0.60399836], np.float32)


def _split3(x64):
    """split x into 3 fp32 parts, hi/mid with 12 zeroed mantissa tail bits"""
    def trunc(v):
        return np.frombuffer((np.frombuffer(np.float32(v).tobytes(), np.uint32)
                              & np.uint32(0xFFFFF000)).tobytes(), np.float32)[0]
    hi = trunc(np.float32(x64))
    mid = trunc(np.float32(x64 - np.float64(hi)))
    lo = np.float32(x64 - np.float64(hi) - np.float64(mid))
    return float(hi), float(mid), float(lo)


CW0, CW1, CW2 = _split3(2 * np.pi)
MAGIC = float(np.float32(1.5 * 2 ** 23))
INV2PI = float(np.float32(1.0 / (2 * np.pi)))


def _expm_skew(X64):
    """expm(triu(X,1) - triu(X,1)^T) via Pade-13 scaling & squaring, float64"""
    Au = np.triu(X64, 1)
    A = Au - Au.T
    nrm = np.linalg.norm(A, 1)
    j = max(0, int(np.ceil(np.log2(max(nrm / 5.4, 2.0 ** -60)))))
    A = A / (2.0 ** j)
    b = [64764752532480000., 32382376266240000., 7771770303897600.,
         1187353796428800., 129060195264000., 10559470521600., 670442572800.,
         33522128640., 1323241920., 40840800., 960960., 16380., 182., 1.]
    I = np.eye(A.shape[0])
    A2 = A @ A; A4 = A2 @ A2; A6 = A2 @ A4
    U = A @ (A6 @ (b[13] * A6 + b[11] * A4 + b[9] * A2)
             + b[7] * A6 + b[5] * A4 + b[3] * A2 + b[1] * I)
    V = (A6 @ (b[12] * A6 + b[10] * A4 + b[8] * A2)
         + b[6] * A6 + b[4] * A4 + b[2] * A2 + b[0] * I)
    E = np.linalg.solve(V - U, V + U)
    for _ in range(j):
        E = E @ E
    return E


def _dap(t, offset, pairs):
    return bass.AP(tensor=t.tensor, offset=offset, ap=[list(p) for p in pairs])


# ---------------- device program ----------------
_NC_CACHE = {}
_LAST = {}


def _build_nc():
    if "nc" in _NC_CACHE:
        return _NC_CACHE["nc"]
    nc = bacc.Bacc("TRN2", target_bir_lowering=False, debug=False,
                   num_devices=NCORES)

    def din(name, shape):
        return nc.dram_tensor(name, shape, dt.float32, kind="ExternalInput").ap()

    def dout(name, shape):
        return nc.dram_tensor(name, shape, dt.float32, kind="ExternalOutput").ap()

    # per-core sharded inputs
    theta_in = din("theta_in", [FC])      # atan2(x_im, x_re) for this shard
    lh_in = din("lh_in", [FC])            # log|x| for this shard
    # shared constants
    m_col = din("m_col", [128, 1])
    negm_col = din("negm_col", [128, 1])
    b_col = din("b_col", [128, 1])
    c_col = din("c_col", [128, 1])
    w_ag = din("w_ag", [128, 128])        # blockdiag8(AG^T)
    w_sum = din("w_sum", [128, 8])        # channel-sum weights
    jrev = din("jrev", [128, 128])        # partition reversal permutation
    t2r_in = din("t2r", [512, N2])
    t2i_in = din("t2i", [512, N2])
    t2ni_in = din("t2ni", [512, N2])
    t1r_in = din("t1r", [N1, N1])
    t1i_in = din("t1i", [N1, N1])
    t1ni_in = din("t1ni", [N1, N1])
    twr_in = din("twr", [N1, N2])
    twi_in = din("twi", [N1, N2])
    tbr_in = din("tbr", [NH])
    tbi_in = din("tbi", [NH])
    # outputs
    out_hre = dout("out_hre", [128, FG])
    out_him = dout("out_him", [128, FG])
    out_h = dout("out_h", [NFFT])
    # internal DRAM
    z_re_d = nc.dram_tensor("z_re_d", [NH], dt.float32).ap()
    z_im_d = nc.dram_tensor("z_im_d", [NH], dt.float32).ap()
    hs_full_re = nc.dram_tensor("hs_full_re", [F], dt.float32).ap()
    hs_full_im = nc.dram_tensor("hs_full_im", [F], dt.float32).ap()
    scr = nc.dram_tensor("scr", [128], dt.float32).ap()

    with tile.TileContext(nc) as tc, ExitStack() as ctx:
        dpool = ctx.enter_context(tc.tile_pool(name="dram", bufs=1, space="DRAM"))
        hs_shard = dpool.tile([2, FC], dt.float32)
        hs_gath = dpool.tile([NCORES, 2, FC], dt.float32, addr_space="Shared")

        cpool = ctx.enter_context(tc.tile_pool(name="consts", bufs=1))
        mc = cpool.tile([128, 1], dt.float32, tag="mc")
        nmc = cpool.tile([128, 1], dt.float32, tag="nmc")
        bc = cpool.tile([128, 1], dt.float32, tag="bc")
        cc = cpool.tile([128, 1], dt.float32, tag="cc")
        hpi = cpool.tile([128, 1], dt.float32, tag="hpi")
        wag = cpool.tile([128, 128], dt.float32, tag="wag")
        wsum = cpool.tile([128, 8], dt.float32, tag="wsum")
        nc.sync.dma_start(mc[:], m_col[:])
        nc.sync.dma_start(nmc[:], negm_col[:])
        nc.sync.dma_start(bc[:], b_col[:])
        nc.sync.dma_start(cc[:], c_col[:])
        nc.vector.memset(hpi[:], float(np.pi / 2))
        nc.sync.dma_start(wag[:], w_ag[:])
        nc.sync.dma_start(wsum[:], w_sum[:])

        spool = ctx.enter_context(tc.tile_pool(name="solve", bufs=1))
        ar = spool.tile([128, FG], dt.float32, tag="ar")    # Dinv real
        bi = spool.tile([128, FG], dt.float32, tag="bi")    # Dinv imag
        z0r = spool.tile([128, FG], dt.float32, tag="z0r")
        z0i = spool.tile([128, FG], dt.float32, tag="z0i")
        z1r = spool.tile([128, FG], dt.float32, tag="z1r")
        z1i = spool.tile([128, FG], dt.float32, tag="z1i")

        # ---- phase precompute ----
        with tc.tile_pool(name="pre", bufs=1) as pp:
            th = pp.tile([128, FG], dt.float32, tag="th")
            lht = pp.tile([128, FG], dt.float32, tag="lht")
            for g in range(NGRP):
                nc.sync.dma_start(th[16 * g:16 * (g + 1), :],
                                  _dap(theta_in, g * FG, [[0, 16], [1, FG]]))
                nc.sync.dma_start(lht[16 * g:16 * (g + 1), :],
                                  _dap(lh_in, g * FG, [[0, 16], [1, FG]]))
            with tc.tile_pool(name="prechunk", bufs=3) as pc:
                # pass A: -sin(phi) -> z0r, cos(phi) -> z0i
                for c0, cl in CHUNKS:
                    sl = slice(c0, c0 + cl)
                    phi = pc.tile([128, CHUNK], dt.float32, tag="w0")
                    nc.vector.tensor_scalar(phi[:, :cl], th[:, sl], mc[:], None,
                                            op0=ALU.mult)
                    kk = pc.tile([128, CHUNK], dt.float32, tag="w1")
                    nc.vector.tensor_scalar(kk[:, :cl], phi[:, :cl], INV2PI,
                                            MAGIC, op0=ALU.mult, op1=ALU.add)
                    nc.vector.tensor_scalar_sub(kk[:, :cl], kk[:, :cl], MAGIC)
                    rr = pc.tile([128, CHUNK], dt.float32, tag="w2")
                    nc.vector.cody_waite_cascade(rr[:, :cl], phi[:, :cl],
                                                 kk[:, :cl], CW0, CW1, CW2)
                    nc.vector.add_range_wrap(rr[:, :cl], rr[:, :cl], 0.0,
                                             float(np.pi), float(2 * np.pi))
                    nc.scalar.activation(z0r[:, sl], rr[:, :cl], AF.Sin,
                                         scale=-1.0)          # -sin(phi)
                    aa = pc.tile([128, CHUNK], dt.float32, tag="w3")
                    nc.scalar.activation(aa[:, :cl], rr[:, :cl], AF.Abs)
                    nc.scalar.activation(z0i[:, sl], aa[:, :cl], AF.Sin,
                                         bias=hpi[:], scale=-1.0)  # cos(phi)
                # pass B: rhoinv = exp(-m*lh); ar = cos*rhoinv; bi = -sin*rhoinv
                for c0, cl in CHUNKS:
                    sl = slice(c0, c0 + cl)
                    rho = pc.tile([128, CHUNK], dt.float32, tag="w0")
                    nc.scalar.activation(rho[:, :cl], lht[:, sl], AF.Exp,
                                         scale=nmc[:])
                    nc.vector.tensor_mul(ar[:, sl], z0i[:, sl], rho[:, :cl])
                    nc.vector.tensor_mul(bi[:, sl], z0r[:, sl], rho[:, :cl])

        # ---- Neumann iteration: z_t = Dinv*(B + AG z_{t-1}), z_0 = Dinv*B ----
        nc.vector.tensor_scalar(z0r[:], ar[:], bc[:], None, op0=ALU.mult)
        nc.vector.tensor_scalar(z0i[:], bi[:], bc[:], None, op0=ALU.mult)
        zs = [(z0r, z0i), (z1r, z1i)]
        with tc.tile_pool(name="nps", bufs=2, space="PSUM") as nps, \
             tc.tile_pool(name="nwk", bufs=3) as nwk:
            for t in range(1, KITER):
                cur, nxt = zs[(t + 1) % 2], zs[t % 2]
                for c0, cl in CHUNKS:
                    sl = slice(c0, c0 + cl)
                    ur = nps.tile([128, CHUNK], dt.float32, tag="ur")
                    ui = nps.tile([128, CHUNK], dt.float32, tag="ui")
                    nc.tensor.matmul(ur[:, :cl], lhsT=wag[:], rhs=cur[0][:, sl],
                                     start=True, stop=True)
                    nc.tensor.matmul(ui[:, :cl], lhsT=wag[:], rhs=cur[1][:, sl],
                                     start=True, stop=True)
                    p = nwk.tile([128, CHUNK], dt.float32, tag="p")
                    q = nwk.tile([128, CHUNK], dt.float32, tag="q")
                    s = nwk.tile([128, CHUNK], dt.float32, tag="s")
                    t2 = nwk.tile([128, CHUNK], dt.float32, tag="t2")
                    # p=(ur+B)*ar ; t2=(ur+B)*bi ; q=bi*ui ; s=ar*ui
                    nc.vector.scalar_tensor_tensor(p[:, :cl], ur[:, :cl], bc[:],
                                                   ar[:, sl], op0=ALU.add,
                                                   op1=ALU.mult)
                    nc.vector.scalar_tensor_tensor(t2[:, :cl], ur[:, :cl], bc[:],
                                                   bi[:, sl], op0=ALU.add,
                                                   op1=ALU.mult)
                    nc.vector.tensor_mul(q[:, :cl], bi[:, sl], ui[:, :cl])
                    nc.vector.tensor_mul(s[:, :cl], ar[:, sl], ui[:, :cl])
                    nc.vector.tensor_sub(nxt[0][:, sl], p[:, :cl], q[:, :cl])
                    nc.vector.tensor_add(nxt[1][:, sl], s[:, :cl], t2[:, :cl])

        zfr, zfi = zs[(KITER - 1) % 2]

        # ---- H = C*z out; Hs = sum_ch H via matmul ----
        hs_re = spool.tile([8, FG], dt.float32, tag="hsr")
        hs_im = spool.tile([8, FG], dt.float32, tag="hsi")
        with tc.tile_pool(name="hps", bufs=2, space="PSUM") as hps, \
             tc.tile_pool(name="hwk", bufs=3) as hwk:
            for c0, cl in CHUNKS:
                sl = slice(c0, c0 + cl)
                hr = hwk.tile([128, CHUNK], dt.float32, tag="hr")
                hi = hwk.tile([128, CHUNK], dt.float32, tag="hi")
                nc.vector.tensor_scalar(hr[:, :cl], zfr[:, sl], cc[:], None,
                                        op0=ALU.mult)
                nc.vector.tensor_scalar(hi[:, :cl], zfi[:, sl], cc[:], None,
                                        op0=ALU.mult)
                nc.sync.dma_start(out_hre[:, sl], hr[:, :cl])
                nc.sync.dma_start(out_him[:, sl], hi[:, :cl])
                pr = hps.tile([8, CHUNK], dt.float32, tag="pr")
                pi = hps.tile([8, CHUNK], dt.float32, tag="pi")
                nc.tensor.matmul(pr[:, :cl], lhsT=wsum[:], rhs=hr[:, :cl],
                                 start=True, stop=True)
                nc.tensor.matmul(pi[:, :cl], lhsT=wsum[:], rhs=hi[:, :cl],
                                 start=True, stop=True)
                nc.vector.tensor_copy(hs_re[:, sl], pr[:, :cl])
                nc.vector.tensor_copy(hs_im[:, sl], pi[:, :cl])
        # store shard [2, FC]: row-major (g, j)
        nc.sync.dma_start(hs_shard[0].rearrange("(g j) -> g j", g=8), hs_re[:, :])
        nc.sync.dma_start(hs_shard[1].rearrange("(g j) -> g j", g=8), hs_im[:, :])

        # ---- AllGather + compaction ----
        nc.gpsimd.collective_compute(
            "AllGather", ALU.bypass, replica_groups=[list(range(NCORES))],
            ins=[hs_shard[:]], outs=[hs_gath[:]],
        )
        with tc.tile_pool(name="cmp", bufs=4) as cmp_p:
            for c in range(NCORES):
                for pl, dst in ((0, hs_full_re), (1, hs_full_im)):
                    bt = cmp_p.tile([8, 3000], dt.float32, tag="bt", name="bt")
                    gsrc = hs_gath[c, pl, :FSTEP].rearrange("(r j) -> r j", r=8)
                    nc.sync.dma_start(bt[:], gsrc)
                    ddst = hs_full_re if pl == 0 else hs_full_im
                    nc.sync.dma_start(
                        _dap(ddst, FSTEP * c, [[3000, 8], [1, 3000]]), bt[:])
            # final bin F-1 = 192000 comes from core 7 offset 24000
            bt2 = cmp_p.tile([1, 1], dt.float32, tag="bt2", name="bt2")
            nc.sync.dma_start(bt2[:], hs_gath[7, 0, FSTEP:FSTEP + 1][None, :])
            nc.sync.dma_start(hs_full_re[F - 1:F][None, :], bt2[:])
            bt3 = cmp_p.tile([1, 1], dt.float32, tag="bt3", name="bt3")
            nc.sync.dma_start(bt3[:], hs_gath[7, 1, FSTEP:FSTEP + 1][None, :])
            nc.sync.dma_start(hs_full_im[F - 1:F][None, :], bt3[:])

        # ---- G build: Z[k] = E[k] + i O[k] (scaled by 2; scale cancels) ----
        with tc.tile_pool(name="gb", bufs=1) as gb, \
             tc.tile_pool(name="gps", bufs=1, space="PSUM") as gps:
            jr = gb.tile([128, 128], dt.float32, tag="jr")
            nc.sync.dma_start(jr[:], jrev[:])
            xr = gb.tile([128, 1500], dt.float32, tag="xr")
            xi = gb.tile([128, 1500], dt.float32, tag="xi")
            tbrt = gb.tile([128, 1500], dt.float32, tag="tbrt")
            tbit = gb.tile([128, 1500], dt.float32, tag="tbit")
            nc.sync.dma_start(xr[:], _dap(hs_full_re, 0, [[1500, 128], [1, 1500]]))
            nc.sync.dma_start(xi[:], _dap(hs_full_im, 0, [[1500, 128], [1, 1500]]))
            nc.sync.dma_start(tbrt[:], _dap(tbr_in, 0, [[1500, 128], [1, 1500]]))
            nc.sync.dma_start(tbit[:], _dap(tbi_in, 0, [[1500, 128], [1, 1500]]))
            # reversed reads: tmp[p,j] = X[1500p + 1500 - j]; rev = J @ tmp
            tmpr = gb.tile([128, 1500], dt.float32, tag="tmpr")
            tmpi = gb.tile([128, 1500], dt.float32, tag="tmpi")
            nc.sync.dma_start(tmpr[:], _dap(hs_full_re, 1500,
                                            [[1500, 128], [-1, 1500]]))
            nc.sync.dma_start(tmpi[:], _dap(hs_full_im, 1500,
                                            [[1500, 128], [-1, 1500]]))
            rvr = gps.tile([128, 1500], dt.float32, tag="rvr")
            rvi = gps.tile([128, 1500], dt.float32, tag="rvi")
            for c0 in range(0, 1500, 512):
                cl = min(512, 1500 - c0)
                nc.tensor.matmul(rvr[:, c0:c0 + cl], lhsT=jr[:],
                                 rhs=tmpr[:, c0:c0 + cl], start=True, stop=True)
                nc.tensor.matmul(rvi[:, c0:c0 + cl], lhsT=jr[:],
                                 rhs=tmpi[:, c0:c0 + cl], start=True, stop=True)
            er = gb.tile([128, 1500], dt.float32, tag="er")
            ei = gb.tile([128, 1500], dt.float32, tag="ei")
            opr = gb.tile([128, 1500], dt.float32, tag="opr")
            opi = gb.tile([128, 1500], dt.float32, tag="opi")
            nc.vector.tensor_add(er[:], xr[:], rvr[:])
            nc.vector.tensor_sub(ei[:], xi[:], rvi[:])
            nc.vector.tensor_sub(opr[:], xr[:], rvr[:])
            nc.vector.tensor_add(opi[:], xi[:], rvi[:])
            our = gb.tile([128, 1500], dt.float32, tag="our")
            oui = gb.tile([128, 1500], dt.float32, tag="oui")
            tq = gb.tile([128, 1500], dt.float32, tag="tq")
            nc.vector.tensor_mul(our[:], tbrt[:], opr[:])
            nc.vector.tensor_mul(tq[:], tbit[:], opi[:])
            nc.vector.tensor_sub(our[:], our[:], tq[:])
            nc.vector.tensor_mul(oui[:], tbrt[:], opi[:])
            nc.vector.tensor_mul(tq[:], tbit[:], opr[:])
            nc.vector.tensor_add(oui[:], oui[:], tq[:])
            nc.vector.tensor_sub(er[:], er[:], oui[:])   # Z_re = E_re - O_im
            nc.vector.tensor_add(ei[:], ei[:], our[:])   # Z_im = E_im + O_re
            nc.sync.dma_start(_dap(z_re_d, 0, [[1500, 128], [1, 1500]]), er[:])
            nc.sync.dma_start(_dap(z_im_d, 0, [[1500, 128], [1, 1500]]), ei[:])

        # ---- FFT stage 2 + twiddle;  Y1[a,d] = sum_b Zmat[a,b] T2[b,d] ----
        BCH = [(0, 128), (128, 128), (256, 128), (384, 116)]
        with tc.tile_pool(name="y2p", bufs=1) as y2p:
            y2 = [(y2p.tile([128, N2], dt.float32, tag=f"y2r{i}", name=f"y2r{i}"),
                   y2p.tile([128, N2], dt.float32, tag=f"y2i{i}", name=f"y2i{i}"))
                  for i in range(3)]
            with tc.tile_pool(name="f2c", bufs=1) as f2c, \
                 tc.tile_pool(name="f2w", bufs=2) as f2w, \
                 tc.tile_pool(name="f2ps", bufs=2, space="PSUM") as f2ps:
                t2t = {}
                for ib, (b0, bn) in enumerate(BCH):
                    for nm, src in (("r", t2r_in), ("i", t2i_in), ("ni", t2ni_in)):
                        tt_ = f2c.tile([128, N2], dt.float32, tag=f"t2{nm}{ib}",
                                       name=f"t2{nm}{ib}")
                        nc.sync.dma_start(tt_[:bn, :], src[b0:b0 + bn, :])
                        t2t[nm, ib] = tt_
                for ac in range(3):
                    y1r = f2ps.tile([128, N2], dt.float32, tag="y1r")
                    y1i = f2ps.tile([128, N2], dt.float32, tag="y1i")
                    for ib, (b0, bn) in enumerate(BCH):
                        zw_r = f2w.tile([128, 128], dt.float32, tag="zwr")
                        zw_i = f2w.tile([128, 128], dt.float32, tag="zwi")
                        nc.sync.dma_start(zw_r[:bn, :],
                                          _dap(z_re_d, 128 * ac + N1 * b0,
                                               [[N1, bn], [1, 128]]))
                        nc.sync.dma_start(zw_i[:bn, :],
                                          _dap(z_im_d, 128 * ac + N1 * b0,
                                               [[N1, bn], [1, 128]]))
                        st = (ib == 0)
                        sp = (ib == len(BCH) - 1)
                        nc.tensor.matmul(y1r[:], lhsT=zw_r[:bn, :],
                                         rhs=t2t["r", ib][:bn, :],
                                         start=st, stop=False)
                        nc.tensor.matmul(y1r[:], lhsT=zw_i[:bn, :],
                                         rhs=t2t["ni", ib][:bn, :],
                                         start=False, stop=sp)
                        nc.tensor.matmul(y1i[:], lhsT=zw_r[:bn, :],
                                         rhs=t2t["i", ib][:bn, :],
                                         start=st, stop=False)
                        nc.tensor.matmul(y1i[:], lhsT=zw_i[:bn, :],
                                         rhs=t2t["r", ib][:bn, :],
                                         start=False, stop=sp)
                    twr_t = f2w.tile([128, N2], dt.float32, tag="twr")
                    twi_t = f2w.tile([128, N2], dt.float32, tag="twi")
                    nc.sync.dma_start(twr_t[:], twr_in[128 * ac:128 * (ac + 1), :])
                    nc.sync.dma_start(twi_t[:], twi_in[128 * ac:128 * (ac + 1), :])
                    y2r, y2i = y2[ac]
                    tq1 = f2w.tile([128, N2], dt.float32, tag="tq1")
                    tq2 = f2w.tile([128, N2], dt.float32, tag="tq2")
                    nc.vector.tensor_mul(tq1[:], twr_t[:], y1r[:])
                    nc.vector.tensor_mul(tq2[:], twi_t[:], y1i[:])
                    nc.vector.tensor_sub(y2r[:], tq1[:], tq2[:])
                    nc.vector.tensor_mul(tq1[:], twr_t[:], y1i[:])
                    nc.vector.tensor_mul(tq2[:], twi_t[:], y1r[:])
                    nc.vector.tensor_add(y2i[:], tq1[:], tq2[:])

            # ---- FFT stage 4: z[c,d] = sum_a T1[a,c] Y2[a,d]; max; output ----
            with tc.tile_pool(name="f4c", bufs=2) as f4c, \
                 tc.tile_pool(name="f4ps", bufs=2, space="PSUM") as f4ps, \
                 tc.tile_pool(name="zzp", bufs=1) as zzp:
                zz = [(zzp.tile([128, N2], dt.float32, tag=f"zzr{i}", name=f"zzr{i}"),
                       zzp.tile([128, N2], dt.float32, tag=f"zzi{i}", name=f"zzi{i}"))
                      for i in range(3)]
                mxt = zzp.tile([128, 1], dt.float32, tag="mxt")
                mxc = zzp.tile([128, 1], dt.float32, tag="mxc")
                for cc_ in range(3):
                    zr_ps = f4ps.tile([128, N2], dt.float32, tag="zr")
                    zi_ps = f4ps.tile([128, N2], dt.float32, tag="zi")
                    for ac in range(3):
                        t1r_b = f4c.tile([128, 128], dt.float32, tag="t1r")
                        t1i_b = f4c.tile([128, 128], dt.float32, tag="t1i")
                        t1ni_b = f4c.tile([128, 128], dt.float32, tag="t1ni")
                        rsl = slice(128 * ac, 128 * (ac + 1))
                        csl = slice(128 * cc_, 128 * (cc_ + 1))
                        nc.sync.dma_start(t1r_b[:], t1r_in[rsl, csl])
                        nc.sync.dma_start(t1i_b[:], t1i_in[rsl, csl])
                        nc.sync.dma_start(t1ni_b[:], t1ni_in[rsl, csl])
                        st = (ac == 0)
                        sp = (ac == 2)
                        y2r, y2i = y2[ac]
                        nc.tensor.matmul(zr_ps[:], lhsT=t1r_b[:], rhs=y2r[:],
                                         start=st, stop=False)
                        nc.tensor.matmul(zr_ps[:], lhsT=t1ni_b[:], rhs=y2i[:],
                                         start=False, stop=sp)
                        nc.tensor.matmul(zi_ps[:], lhsT=t1i_b[:], rhs=y2r[:],
                                         start=st, stop=False)
                        nc.tensor.matmul(zi_ps[:], lhsT=t1r_b[:], rhs=y2i[:],
                                         start=False, stop=sp)
                    zzr, zzi = zz[cc_]
                    nc.vector.tensor_copy(zzr[:], zr_ps[:])
                    nc.vector.tensor_copy(zzi[:], zi_ps[:])
                    for ip, pl in enumerate((zzr, zzi)):
                        red = f4c.tile([128, 1], dt.float32, tag="red")
                        nc.vector.tensor_reduce(red[:], pl[:], axis=AX.X,
                                                op=ALU.max,
                                                apply_absolute_value=True)
                        if cc_ == 0 and ip == 0:
                            nc.vector.tensor_copy(mxt[:], red[:])
                        else:
                            nc.vector.tensor_max(mxt[:], mxt[:], red[:])
                # partition reduce via DRAM roundtrip, then broadcast 1/max
                nc.sync.dma_start(scr[:], mxt[:, 0])
                mrow = f4c.tile([1, 128], dt.float32, tag="mrow")
                nc.sync.dma_start(mrow[:], scr[None, :])
                m1 = f4c.tile([1, 1], dt.float32, tag="m1")
                nc.vector.tensor_reduce(m1[:], mrow[:], axis=AX.X, op=ALU.max)
                rc = f4c.tile([1, 1], dt.float32, tag="rc")
                nc.vector.reciprocal(rc[:], m1[:])
                nc.sync.dma_start(scr[:1], rc[:, 0])
                nc.sync.dma_start(mxc[:], _dap(scr, 0, [[0, 128], [1, 1]]))
                for cc_ in range(3):
                    zzr, zzi = zz[cc_]
                    hint = f4c.tile([128, 2 * N2], dt.float32, tag="hint",
                                    name="hint")
                    nc.vector.tensor_scalar(
                        hint.rearrange("p (d two) -> p d two", two=2)[:, :, 0],
                        zzr[:], mxc[:], None, op0=ALU.mult)
                    nc.vector.tensor_scalar(
                        hint.rearrange("p (d two) -> p d two", two=2)[:, :, 1],
                        zzi[:], mxc[:], None, op0=ALU.mult)
                    nc.sync.dma_start(_dap(out_h, 2 * N2 * 128 * cc_,
                                           [[2 * N2, 128], [1, 2 * N2]]),
                                      hint[:])

    nc.compile()
    _NC_CACHE["nc"] = nc
    return nc


# ---------------- host side ----------------
def _host_constants(AG):
    f32 = np.float32
    c = {}
    ch = np.arange(128) % 16
    c["m_col"] = M_DELAYS[ch].reshape(128, 1).copy()
    c["negm_col"] = (-M_DELAYS[ch]).reshape(128, 1).copy()
    wag = np.zeros((128, 128), f32)
    for g in range(8):
        wag[16 * g:16 * (g + 1), 16 * g:16 * (g + 1)] = AG.T
    c["w_ag"] = wag
    wsum = np.zeros((128, 8), f32)
    for g in range(8):
        wsum[16 * g:16 * (g + 1), g] = 1.0
    c["w_sum"] = wsum
    c["jrev"] = np.eye(128, dtype=f32)[:, ::-1].copy()
    b, d = np.meshgrid(np.arange(512), np.arange(N2), indexing="ij")
    ang = 2 * np.pi * ((b * d) % N2) / N2
    mask = (b < N2)
    c["t2r"] = (np.cos(ang) * mask).astype(f32)
    c["t2i"] = (np.sin(ang) * mask).astype(f32)
    c["t2ni"] = (-np.sin(ang) * mask).astype(f32)
    a, cg = np.meshgrid(np.arange(N1), np.arange(N1), indexing="ij")
    ang = 2 * np.pi * ((a * cg) % N1) / N1
    c["t1r"] = (np.cos(ang) / NH).astype(f32)
    c["t1i"] = (np.sin(ang) / NH).astype(f32)
    c["t1ni"] = (-np.sin(ang) / NH).astype(f32)
    a, d = np.meshgrid(np.arange(N1), np.arange(N2), indexing="ij")
    ang = 2 * np.pi * (a.astype(np.float64) * d) / NH
    c["twr"] = np.cos(ang).astype(f32)
    c["twi"] = np.sin(ang).astype(f32)
    k = np.arange(NH)
    ang = 2 * np.pi * k / NFFT
    c["tbr"] = np.cos(ang).astype(f32)
    c["tbi"] = np.sin(ang).astype(f32)
    return c


def kernel(x_real, x_imag, B, C, X):
    f32 = np.float32
    x_real = np.asarray(x_real, f32)
    x_imag = np.asarray(x_imag, f32)
    A = _expm_skew(np.asarray(X, np.float64))
    AG = (A * GAMMA_F32.astype(np.float64)[None, :]).astype(f32)
    theta = np.arctan2(x_imag, x_real).astype(f32)
    lh = np.log(np.hypot(x_real, x_imag)).astype(f32)
    Bv = np.asarray(B, f32).reshape(N)
    Cv = np.asarray(C, f32).reshape(N)

    consts = _host_constants(AG)
    ch = np.arange(128) % 16
    consts["b_col"] = Bv[ch].reshape(128, 1).copy()
    consts["c_col"] = Cv[ch].reshape(128, 1).copy()

    idx = np.arange(FC)
    in_maps = []
    for c in range(NCORES):
        fidx = np.clip(FSTEP * c + idx, 0, F - 1)
        m = dict(consts)
        m["theta_in"] = theta[fidx].copy()
        m["lh_in"] = lh[fidx].copy()
        in_maps.append(m)

    nc = _build_nc()
    if os.environ.get("DFDN_SIM") == "1":
        from concourse.bass_interp import MultiCoreSim
        sim = MultiCoreSim(nc, num_cores=NCORES)
        for i in range(NCORES):
            for k, v in in_maps[i].items():
                sim.cores[i].tensor(k)[:] = v
        sim.simulate(check_with_hw=False)
        outs = ["out_hre", "out_him", "out_h"]
        results = [{nm: np.array(sim.cores[i].tensor(nm)) for nm in outs}
                   for i in range(NCORES)]

        class _R:
            pass
        br = _R()
        br.results = results
        br.exec_time_ns = None
    else:
        br = run_bass_kernel_spmd(nc, in_maps, core_ids=list(range(NCORES)),
                                  trace=os.environ.get("DFDN_TRACE") == "1")
    _LAST["br"] = br

    H = np.empty((F, N), np.complex64)
    for c in range(NCORES):
        ln = FSTEP if c < NCORES - 1 else FSTEP + 1
        hr = br.results[c]["out_hre"].reshape(8, 16, FG).transpose(0, 2, 1).reshape(FC, 16)
        hi = br.results[c]["out_him"].reshape(8, 16, FG).transpose(0, 2, 1).reshape(FC, 16)
        H[FSTEP * c:FSTEP * c + ln] = (hr[:ln] + 1j * hi[:ln]).astype(np.complex64)
    h = br.results[0]["out_h"].astype(f32)
    return H, h
